# revision 34
# baseline (speedup 1.0000x reference)
"""Linear-attention (sparse_attention) Trainium2 Bass kernel.

Problem: nn_Attention_Linear_25709674234652
  B=4, S=8192, D=1024, H=16 heads, HD=64, AD=64 (approx dim), EPS=1e-6

  qkv = x @ W_qkv.T (+0)          [B,S,3D]
  per head: pQ = Q @ W_p.T, pK = K @ W_p.T, phi(u) = sqrt(1+u^2)
  KTV = phi_K^T @ V  [AD,HD],  k_sum = sum_s phi_K
  out = (phi_Q @ KTV) / (phi_Q @ k_sum + eps)

Sharding: 8 cores = 4 batches x 2 head-groups (8 heads each). Each core is
fully independent (no collectives).

Host-side tricks:
  - W_p @ W_q and W_p @ W_k are folded into single projection matrices, so
    the device computes pQ / pK directly from x; Q and K never exist.
  - x is passed transposed (x^T) so the contraction dim D is already on
    partitions; no on-chip transposes anywhere.
  - inputs cast to bf16 on host (fp32 matmul on TRN2 costs ~4x bf16);
    fp32 accumulation in PSUM. Validated rel err ~3.4e-3.

Device structure:
  - pass A (per 512-col s-block): pQ^T feature-major -> phi -> bf16
    phi_Q kept RESIDENT in SBUF (8 MiB); pK|V row-major -> phi(pK), V
    -> KTV accumulated over all of S in PSUM (k_sum rides along as a
    ones-column appended to V). KTV matmuls are emitted ~2 blocks late
    so the in-order PE never waits on the ACT phi chain.
  - pass B (per 128-row s-block): one N=130 matmul per head-pair against
    block-diagonal KTV + k_sum columns (num and den in one shot),
    reciprocal + broadcast multiply on DVE, store fp32. The last 5
    s-blocks' pQ matmul groups are deferred into pass B to fill its
    otherwise-idle PE (they have no KTV dependency).

Measured on HW: ~412 us exec (core 0 NTFF), rel err 3.4e-3 vs fp32 ref.
PE-bound: bf16 projection floor is ~330 us of the ~390 us PE-busy time.
"""

import numpy as np
import ml_dtypes

import concourse.bass as bass
import concourse.tile as tile
from concourse import bacc, mybir
from concourse.bass_utils import run_bass_kernel_spmd

# ---- problem dims (hardcoded per spec) ----
B, S, D = 4, 8192, 1024
H, HD, AD = 16, 64, 64
EPS = 1e-6
NCORES = 8
HG = H // 2          # heads per core = 8
CH = HG * AD         # phi channels per core = 512
CV = HG * HD         # value channels per core = 512
P = 128
NKD = D // P         # 8 contraction tiles
SB = 512             # pass-A s-block
NSB = S // SB        # 16
NPAIR = CH // P      # 4 head-pairs per core
NB2 = S // P         # 64 pass-B s-blocks
F32 = mybir.dt.float32
BF16 = mybir.dt.bfloat16

_CACHE = {}
LAST_RESULTS = None  # BassKernelResults of most recent run (for profiling)


def _build_nc():
    nc = bacc.Bacc()
    AF = mybir.ActivationFunctionType

    xt = nc.dram_tensor("xt", [D, S], BF16, kind="ExternalInput")
    wqp = nc.dram_tensor("wqp", [D, CH], BF16, kind="ExternalInput")
    wkv = nc.dram_tensor("wkv", [D, CH + CV], BF16, kind="ExternalInput")
    out = nc.dram_tensor("out", [S, CV], F32, kind="ExternalOutput")

    xt_r = xt.rearrange("(kd p) s -> p kd s", p=P)
    wqp_r = wqp.rearrange("(kd p) c -> p kd c", p=P)
    wkv_r = wkv.rearrange("(kd p) c -> p kd c", p=P)

    with tile.TileContext(nc) as tc:
        with (
            tc.tile_pool(name="singles", bufs=1) as singles,
            tc.tile_pool(name="xload", bufs=2) as xload,
            tc.tile_pool(name="sqp", bufs=3) as sqpool,
            tc.tile_pool(name="phikp", bufs=6) as phikpool,
            tc.tile_pool(name="vp", bufs=6) as vpool,
        ):
            # startup critical path: per-kd DMAs so the first matmul (needs
            # only x[kd0] + wqp[kd0]) starts after ~0.4 MiB, not ~4 MiB
            def load_x_block(sb):
                tiles = []
                for kd in range(NKD):
                    xt_kd = xload.tile([P, SB], BF16, tag=f"x{kd}",
                                       name=f"x_{sb}_{kd}")
                    nc.sync.dma_start(
                        out=xt_kd, in_=xt_r[:, kd, sb * SB:(sb + 1) * SB]
                    )
                    tiles.append(xt_kd)
                return tiles

            # interleave x[kd] / wqp[kd] so the kd=0 matmul's deps drain first
            w_qp = singles.tile([P, NKD, CH], BF16)
            w_kv = singles.tile([P, NKD, CH + CV], BF16)
            x_first = []
            for kd in range(NKD):
                xt_kd = xload.tile([P, SB], BF16, tag=f"x{kd}", name=f"x_0_{kd}")
                nc.sync.dma_start(out=xt_kd, in_=xt_r[:, kd, 0:SB])
                x_first.append(xt_kd)
                nc.sync.dma_start(out=w_qp[:, kd], in_=wqp_r[:, kd])
            for kd in range(NKD):
                nc.sync.dma_start(out=w_kv[:, kd], in_=wkv_r[:, kd])
            # phi_Q^T resident: [128, 4 q-tiles, S] bf16 = 64 KiB/partition
            phiq_sb = singles.tile([P, NPAIR, S], BF16)

            with (
                tc.tile_pool(name="ps_q", bufs=2, space="PSUM") as ps_q,
                tc.tile_pool(name="ps_k", bufs=2, space="PSUM") as ps_k,
                tc.tile_pool(name="ps_v", bufs=2, space="PSUM") as ps_v,
                tc.tile_pool(name="ps_acc", bufs=1, space="PSUM") as ps_acc,
            ):
                # persistent accumulators, live across the whole pass.
                # col 128 of each pair block accumulates k_sum (ones column
                # appended to V), so no separate ksum matmuls are needed.
                # 2 pairs x 129 cols = 1032 B < 2 KiB, fits one bank.
                PV1 = P + 1
                ktv_ps_ab = [
                    ps_acc.tile([P, 2, PV1], F32, tag=f"ktv{i}", name=f"ktv{i}")
                    for i in range(2)
                ]

                pending = []

                def emit_ktv(phik_t, v_t, idx):
                    first = idx == 0
                    last = idx == 4 * NSB - 1
                    for pr in range(NPAIR):
                        # [128s x 128a].T @ [128s x 129(v|1)] -> a-pair x (v|ksum)
                        # off-diagonal 64x64 blocks are cross-head garbage,
                        # masked out when copying to SBUF.
                        nc.tensor.matmul(
                            ktv_ps_ab[pr // 2][:, pr % 2, :],
                            phik_t[:, pr * P:(pr + 1) * P],
                            v_t[:, pr, :],
                            start=(first and pr % 2 == 0),
                            stop=(last and pr % 2 == 1),
                        )

                def emit_pq(x_t, sb, pool):
                    # pQ^T feature-major + phi -> resident bf16
                    for qt in range(NPAIR):
                        pq_ps = pool.tile([P, SB], F32, tag="pq",
                                          name=f"pq_{sb}_{qt}")
                        for kd in range(NKD):
                            nc.tensor.matmul(
                                pq_ps,
                                w_qp[:, kd, qt * P:(qt + 1) * P],
                                x_t[kd],
                                start=(kd == 0),
                                stop=(kd == NKD - 1),
                            )
                        sq_t = sqpool.tile([P, SB], F32, tag="sq_q")
                        nc.scalar.square(sq_t, pq_ps)
                        nc.scalar.activation(
                            phiq_sb[:, qt, sb * SB:(sb + 1) * SB],
                            sq_t, AF.Sqrt, bias=1.0,
                        )

                # the last QSHIFT blocks' pQ groups are deferred into pass B
                # (no KTV dependency) to fill pass B's otherwise-idle PE
                QSHIFT = 5
                QS0 = NSB - QSHIFT
                for sb in range(NSB):
                    x_t = x_first if sb == 0 else load_x_block(sb)
                    if sb < QS0:
                        emit_pq(x_t, sb, ps_q)
                    # ---- row-major pK | V + phi + KTV/ksum accumulate ----
                    for st in range(4):
                        pk_ps = ps_k.tile([P, CH], F32, tag="pk")
                        v_ps = ps_v.tile([P, CV], F32, tag="v")
                        for kd in range(NKD):
                            lhsT = x_t[kd][:, st * P:(st + 1) * P]
                            nc.tensor.matmul(
                                pk_ps, lhsT, w_kv[:, kd, :CH],
                                start=(kd == 0), stop=(kd == NKD - 1),
                            )
                            nc.tensor.matmul(
                                v_ps, lhsT, w_kv[:, kd, CH:],
                                start=(kd == 0), stop=(kd == NKD - 1),
                            )
                        sqk_t = sqpool.tile([P, CH], F32, tag="sq_k")
                        nc.scalar.square(sqk_t, pk_ps)
                        phik_t = phikpool.tile([P, CH], BF16, tag="phik")
                        nc.scalar.activation(phik_t, sqk_t, AF.Sqrt, bias=1.0)
                        # V pairs with a ones column appended (k_sum rides the
                        # KTV matmul as output column 128)
                        v_t = vpool.tile([P, NPAIR, P + 1], BF16, tag="vsb")
                        nc.vector.tensor_copy(
                            out=v_t[:, :, 0:P],
                            in_=v_ps[:, :].rearrange("p (q v) -> p q v", v=P),
                        )
                        nc.vector.memset(v_t[:, :, P:P + 1], 1.0)
                        pending.append((phik_t, v_t, sb * 4 + st))
                        # defer KTV emission ~3 blocks so PE never waits on phi
                        while len(pending) > 3:
                            emit_ktv(*pending.pop(0))
                for item in pending:
                    emit_ktv(*item)
                pending.clear()

                # ---- KTV -> block-diag SBUF (bf16), ksum in cols 128-129 ----
                # rhs_all[:, pr] = [ktv_bd (128) | ksum_h0 col | ksum_h1 col]
                # so pass B's den rides the same matmul as num (N=130).
                rhs_all = singles.tile([P, NPAIR, P + 2], BF16)
                nc.vector.memset(rhs_all, 0.0)
                HA = AD  # 64
                for pr in range(NPAIR):
                    kps = ktv_ps_ab[pr // 2][:, pr % 2, :]
                    nc.vector.tensor_copy(
                        out=rhs_all[0:HA, pr, 0:HA], in_=kps[0:HA, 0:HA]
                    )
                    nc.vector.tensor_copy(
                        out=rhs_all[HA:P, pr, HA:P], in_=kps[HA:P, HA:P]
                    )
                    nc.vector.tensor_copy(
                        out=rhs_all[0:HA, pr, P:P + 1], in_=kps[0:HA, P:P + 1]
                    )
                    nc.vector.tensor_copy(
                        out=rhs_all[HA:P, pr, P + 1:P + 2], in_=kps[HA:P, P:P + 1]
                    )

            # ---- pass B: numerator / denominator / divide / store ----
            with (
                tc.tile_pool(name="ps_nd", bufs=3, space="PSUM") as ps_nd,
                tc.tile_pool(name="ps_q2", bufs=2, space="PSUM") as ps_q2,
                tc.tile_pool(name="bwork", bufs=4) as bwork,
                tc.tile_pool(name="bout", bufs=4) as bout,
            ):
                NDW = P + 2  # num (128) + den (2) columns per pair
                # prefetch all deferred-pQ x blocks up front (xq pool holds 5)
                xq_blocks = []
                for j in range(QSHIFT):
                    sbq = QS0 + j
                    tiles = []
                    for kd in range(NKD):
                        xt_kd = xload.tile([P, SB], BF16, tag=f"xq{kd}",
                                           name=f"xq_{sbq}_{kd}", bufs=QSHIFT)
                        nc.sync.dma_start(
                            out=xt_kd, in_=xt_r[:, kd, sbq * SB:(sbq + 1) * SB]
                        )
                        tiles.append(xt_kd)
                    xq_blocks.append(tiles)
                for sb2 in range(NB2):
                    # interleave the deferred pQ groups early in pass B, well
                    # before their consumers (blocks 4*QS0 onward)
                    if sb2 % 10 == 0 and sb2 // 10 < QSHIFT:
                        j = sb2 // 10
                        emit_pq(xq_blocks[j], QS0 + j, ps_q2)
                    # two psum tiles of 2 pairs each: 2*130 f32 = 1040 B/bank
                    nds = [
                        ps_nd.tile([P, 2, NDW], F32, tag=f"nd{i}",
                                   name=f"nd{i}_{sb2}")
                        for i in range(2)
                    ]
                    for pr in range(NPAIR):
                        nc.tensor.matmul(
                            nds[pr // 2][:, pr % 2, :],
                            phiq_sb[:, pr, sb2 * P:(sb2 + 1) * P],
                            rhs_all[:, pr, :],
                            start=(pr % 2 == 0), stop=(pr % 2 == 1),
                        )
                    # rec = 1/(den+eps). den >= 64*8192 (phi >= 1 everywhere),
                    # so EPS=1e-6 is ~12 orders below den and vanishes in fp32
                    # rounding — skip the eps add, reciprocal straight from PSUM.
                    rec = bwork.tile([P, 2, 2, 2], F32, tag="rec")
                    for i in range(2):
                        nc.vector.reciprocal(rec[:, i], nds[i][:, :, P:P + 2])
                    o_t = bout.tile([P, 2 * NPAIR, HD], F32, tag="o")
                    # broadcast multiply: out[s, h, v] = num * rec[s, h]
                    # one 4D-AP op per nd tile (2 pairs each)
                    for i in range(2):
                        nc.vector.tensor_tensor(
                            o_t[:, 4 * i:4 * i + 4, :].rearrange(
                                "p (q j) v -> p q j v", q=2
                            ),
                            nds[i][:, :, 0:P].rearrange(
                                "p q (j v) -> p q j v", v=HD
                            ),
                            rec[:, i, :, :, None].to_broadcast((P, 2, 2, HD)),
                            mybir.AluOpType.mult,
                        )
                    nc.sync.dma_start(
                        out=out[sb2 * P:(sb2 + 1) * P, :],
                        in_=o_t[:, :, :].rearrange("p h v -> p (h v)"),
                    )
    nc.finalize()
    return nc


def _get_nc():
    if "nc" not in _CACHE:
        _CACHE["nc"] = _build_nc()
    return _CACHE["nc"]


def _prep_inputs(x, W_qkv, b_qkv, W_p, b_p):
    """Host-side sharding + weight folding (fp64 fold, bf16 shipping).
    Biases are zero by construction in setup_inputs(); the fold keeps the
    zero bias exact."""
    x = np.asarray(x, dtype=np.float32)
    W_qkv = np.asarray(W_qkv, dtype=np.float32)
    W_p = np.asarray(W_p, dtype=np.float32)
    bf16 = ml_dtypes.bfloat16

    Wq = W_qkv[0:D]
    Wk = W_qkv[D:2 * D]
    Wv = W_qkv[2 * D:3 * D]
    Wp64 = W_p.astype(np.float64)

    xt_b = [np.ascontiguousarray(x[b].T.astype(bf16)) for b in range(B)]

    in_maps = []
    for core in range(NCORES):
        b = core % B
        g = core // B
        rows = slice(g * CV, (g + 1) * CV)
        Wq_g = Wq[rows].astype(np.float64).reshape(HG, HD, D)
        Wk_g = Wk[rows].astype(np.float64).reshape(HG, HD, D)
        # fold the shared AD-projection into the qkv projection
        wqp_g = np.einsum("ah,ghd->gad", Wp64, Wq_g).reshape(CH, D)
        wkp_g = np.einsum("ah,ghd->gad", Wp64, Wk_g).reshape(CH, D)
        wqpT = np.ascontiguousarray(wqp_g.T.astype(bf16))
        wkvT = np.ascontiguousarray(
            np.concatenate([wkp_g.T.astype(np.float32),
                            Wv[rows].T.astype(np.float32)], axis=1).astype(bf16)
        )
        in_maps.append({"xt": xt_b[b], "wqp": wqpT, "wkv": wkvT})
    return in_maps


def kernel(x, W_qkv, b_qkv, W_p, b_p):
    global LAST_RESULTS
    in_maps = _prep_inputs(x, W_qkv, b_qkv, W_p, b_p)
    res = run_bass_kernel_spmd(_get_nc(), in_maps, core_ids=list(range(NCORES)))
    LAST_RESULTS = res
    out_full = np.empty((B, S, D), np.float32)
    for core in range(NCORES):
        b = core % B
        g = core // B
        out_full[b, :, g * CV:(g + 1) * CV] = res.results[core]["out"]
    return out_full


# revision 36
# speedup vs baseline: 1.0275x; 1.0275x over previous
"""Linear-attention (sparse_attention) Trainium2 Bass kernel.

Problem: nn_Attention_Linear_25709674234652
  B=4, S=8192, D=1024, H=16 heads, HD=64, AD=64 (approx dim), EPS=1e-6

  qkv = x @ W_qkv.T (+0)          [B,S,3D]
  per head: pQ = Q @ W_p.T, pK = K @ W_p.T, phi(u) = sqrt(1+u^2)
  KTV = phi_K^T @ V  [AD,HD],  k_sum = sum_s phi_K
  out = (phi_Q @ KTV) / (phi_Q @ k_sum + eps)

Sharding: 8 cores = 4 batches x 2 head-groups (8 heads each). Each core is
fully independent (no collectives).

Host-side tricks:
  - W_p @ W_q and W_p @ W_k are folded into single projection matrices, so
    the device computes pQ / pK directly from x; Q and K never exist.
  - x is passed transposed (x^T) so the contraction dim D is already on
    partitions; no on-chip transposes anywhere.
  - inputs cast to bf16 on host (fp32 matmul on TRN2 costs ~4x bf16);
    fp32 accumulation in PSUM. Validated rel err ~3.4e-3.

Device structure:
  - pass A (per 512-col s-block): pQ^T feature-major -> phi -> bf16
    phi_Q kept RESIDENT in SBUF (8 MiB); pK|V row-major -> phi(pK), V
    -> KTV accumulated over all of S in PSUM (k_sum rides along as a
    ones-column appended to V). KTV matmuls are emitted ~2 blocks late
    so the in-order PE never waits on the ACT phi chain.
  - pass B (per 128-row s-block): one N=130 matmul per head-pair against
    block-diagonal KTV + k_sum columns (num and den in one shot),
    reciprocal + broadcast multiply on DVE, store fp32. The last 5
    s-blocks' pQ matmul groups are deferred into pass B to fill its
    otherwise-idle PE (they have no KTV dependency).

Measured on HW: ~412 us exec (core 0 NTFF), rel err 3.4e-3 vs fp32 ref.
PE-bound: bf16 projection floor is ~330 us of the ~390 us PE-busy time.
"""

import numpy as np
import ml_dtypes

import concourse.bass as bass
import concourse.tile as tile
from concourse import bacc, mybir
from concourse.bass_utils import run_bass_kernel_spmd

# ---- problem dims (hardcoded per spec) ----
B, S, D = 4, 8192, 1024
H, HD, AD = 16, 64, 64
EPS = 1e-6
NCORES = 8
HG = H // 2          # heads per core = 8
CH = HG * AD         # phi channels per core = 512
CV = HG * HD         # value channels per core = 512
P = 128
NKD = D // P         # 8 contraction tiles
SB = 512             # pass-A s-block
NSB = S // SB        # 16
NPAIR = CH // P      # 4 head-pairs per core
NB2 = S // P         # 64 pass-B s-blocks
F32 = mybir.dt.float32
BF16 = mybir.dt.bfloat16

_CACHE = {}
LAST_RESULTS = None  # BassKernelResults of most recent run (for profiling)


def _build_nc():
    nc = bacc.Bacc()
    AF = mybir.ActivationFunctionType

    xt = nc.dram_tensor("xt", [D, S], BF16, kind="ExternalInput")
    wqp = nc.dram_tensor("wqp", [D, CH], BF16, kind="ExternalInput")
    wkv = nc.dram_tensor("wkv", [D, CH + CV], BF16, kind="ExternalInput")
    out = nc.dram_tensor("out", [S, CV], F32, kind="ExternalOutput")

    xt_r = xt.rearrange("(kd p) s -> p kd s", p=P)
    wqp_r = wqp.rearrange("(kd p) c -> p kd c", p=P)
    wkv_r = wkv.rearrange("(kd p) c -> p kd c", p=P)

    with tile.TileContext(nc) as tc:
        with (
            tc.tile_pool(name="singles", bufs=1) as singles,
            tc.tile_pool(name="xload", bufs=2) as xload,
            tc.tile_pool(name="sqp", bufs=3) as sqpool,
            tc.tile_pool(name="phikp", bufs=6) as phikpool,
            tc.tile_pool(name="vp", bufs=6) as vpool,
        ):
            # startup critical path: per-kd DMAs so the first matmul (needs
            # only x[kd0] + wqp[kd0]) starts after ~0.4 MiB, not ~4 MiB
            def load_x_block(sb):
                tiles = []
                for kd in range(NKD):
                    xt_kd = xload.tile([P, SB], BF16, tag=f"x{kd}",
                                       name=f"x_{sb}_{kd}")
                    nc.sync.dma_start(
                        out=xt_kd, in_=xt_r[:, kd, sb * SB:(sb + 1) * SB]
                    )
                    tiles.append(xt_kd)
                return tiles

            # interleave x[kd] / wqp[kd] so the kd=0 matmul's deps drain first
            w_qp = singles.tile([P, NKD, CH], BF16)
            w_kv = singles.tile([P, NKD, CH + CV], BF16)
            x_first = []
            for kd in range(NKD):
                xt_kd = xload.tile([P, SB], BF16, tag=f"x{kd}", name=f"x_0_{kd}")
                nc.sync.dma_start(out=xt_kd, in_=xt_r[:, kd, 0:SB])
                x_first.append(xt_kd)
                nc.sync.dma_start(out=w_qp[:, kd], in_=wqp_r[:, kd])
            for kd in range(NKD):
                nc.sync.dma_start(out=w_kv[:, kd], in_=wkv_r[:, kd])
            # phi_Q^T resident: [128, 4 q-tiles, S] bf16 = 64 KiB/partition
            phiq_sb = singles.tile([P, NPAIR, S], BF16)

            with (
                tc.tile_pool(name="ps_q", bufs=2, space="PSUM") as ps_q,
                tc.tile_pool(name="ps_k", bufs=2, space="PSUM") as ps_k,
                tc.tile_pool(name="ps_v", bufs=2, space="PSUM") as ps_v,
                tc.tile_pool(name="ps_acc", bufs=1, space="PSUM") as ps_acc,
            ):
                # persistent accumulators, live across the whole pass.
                # col 128 of each pair block accumulates k_sum (ones column
                # appended to V), so no separate ksum matmuls are needed.
                # 2 pairs x 129 cols = 1032 B < 2 KiB, fits one bank.
                PV1 = P + 1
                ktv_ps_ab = [
                    ps_acc.tile([P, 2, PV1], F32, tag=f"ktv{i}", name=f"ktv{i}")
                    for i in range(2)
                ]

                pending = []

                def emit_ktv(phik_t, v_t, idx):
                    first = idx == 0
                    last = idx == 4 * NSB - 1
                    for pr in range(NPAIR):
                        # [128s x 128a].T @ [128s x 129(v|1)] -> a-pair x (v|ksum)
                        # off-diagonal 64x64 blocks are cross-head garbage,
                        # masked out when copying to SBUF.
                        nc.tensor.matmul(
                            ktv_ps_ab[pr // 2][:, pr % 2, :],
                            phik_t[:, pr * P:(pr + 1) * P],
                            v_t[:, pr, :],
                            start=(first and pr % 2 == 0),
                            stop=(last and pr % 2 == 1),
                        )

                def emit_pq_qt(x_t, sb, qt, pool):
                    # one pQ^T q-tile: matmul group + phi -> resident bf16
                    pq_ps = pool.tile([P, SB], F32, tag="pq",
                                      name=f"pq_{sb}_{qt}")
                    for kd in range(NKD):
                        nc.tensor.matmul(
                            pq_ps,
                            w_qp[:, kd, qt * P:(qt + 1) * P],
                            x_t[kd],
                            start=(kd == 0),
                            stop=(kd == NKD - 1),
                        )
                    sq_t = sqpool.tile([P, SB], F32, tag="sq_q")
                    nc.scalar.square(sq_t, pq_ps)
                    nc.scalar.activation(
                        phiq_sb[:, qt, sb * SB:(sb + 1) * SB],
                        sq_t, AF.Sqrt, bias=1.0,
                    )

                def emit_pq(x_t, sb, pool):
                    for qt in range(NPAIR):
                        emit_pq_qt(x_t, sb, qt, pool)

                # the last QSHIFT blocks' pQ groups are deferred into pass B
                # (no KTV dependency): spread over pass B's TAIL at qt-group
                # granularity to keep the PE dense there — pass B's bursty
                # pattern otherwise lets HAM re-throttle the PE to 1.2 GHz
                QSHIFT = 7
                QS0 = NSB - QSHIFT
                for sb in range(NSB):
                    x_t = x_first if sb == 0 else load_x_block(sb)
                    if sb < QS0:
                        emit_pq(x_t, sb, ps_q)
                    # ---- row-major pK | V + phi + KTV/ksum accumulate ----
                    for st in range(4):
                        pk_ps = ps_k.tile([P, CH], F32, tag="pk")
                        v_ps = ps_v.tile([P, CV], F32, tag="v")
                        for kd in range(NKD):
                            lhsT = x_t[kd][:, st * P:(st + 1) * P]
                            nc.tensor.matmul(
                                pk_ps, lhsT, w_kv[:, kd, :CH],
                                start=(kd == 0), stop=(kd == NKD - 1),
                            )
                            nc.tensor.matmul(
                                v_ps, lhsT, w_kv[:, kd, CH:],
                                start=(kd == 0), stop=(kd == NKD - 1),
                            )
                        sqk_t = sqpool.tile([P, CH], F32, tag="sq_k")
                        nc.scalar.square(sqk_t, pk_ps)
                        phik_t = phikpool.tile([P, CH], BF16, tag="phik")
                        nc.scalar.activation(phik_t, sqk_t, AF.Sqrt, bias=1.0)
                        # V pairs with a ones column appended (k_sum rides the
                        # KTV matmul as output column 128)
                        v_t = vpool.tile([P, NPAIR, P + 1], BF16, tag="vsb")
                        nc.vector.tensor_copy(
                            out=v_t[:, :, 0:P],
                            in_=v_ps[:, :].rearrange("p (q v) -> p q v", v=P),
                        )
                        nc.vector.memset(v_t[:, :, P:P + 1], 1.0)
                        pending.append((phik_t, v_t, sb * 4 + st))
                        # defer KTV emission ~3 blocks so PE never waits on phi
                        while len(pending) > 3:
                            emit_ktv(*pending.pop(0))
                for item in pending:
                    emit_ktv(*item)
                pending.clear()

                # ---- KTV -> block-diag SBUF (bf16), ksum in cols 128-129 ----
                # rhs_all[:, pr] = [ktv_bd (128) | ksum_h0 col | ksum_h1 col]
                # so pass B's den rides the same matmul as num (N=130).
                rhs_all = singles.tile([P, NPAIR, P + 2], BF16)
                nc.vector.memset(rhs_all, 0.0)
                HA = AD  # 64
                for pr in range(NPAIR):
                    kps = ktv_ps_ab[pr // 2][:, pr % 2, :]
                    nc.vector.tensor_copy(
                        out=rhs_all[0:HA, pr, 0:HA], in_=kps[0:HA, 0:HA]
                    )
                    nc.vector.tensor_copy(
                        out=rhs_all[HA:P, pr, HA:P], in_=kps[HA:P, HA:P]
                    )
                    nc.vector.tensor_copy(
                        out=rhs_all[0:HA, pr, P:P + 1], in_=kps[0:HA, P:P + 1]
                    )
                    nc.vector.tensor_copy(
                        out=rhs_all[HA:P, pr, P + 1:P + 2], in_=kps[HA:P, P:P + 1]
                    )

            # ---- pass B: numerator / denominator / divide / store ----
            with (
                tc.tile_pool(name="ps_nd", bufs=3, space="PSUM") as ps_nd,
                tc.tile_pool(name="ps_q2", bufs=2, space="PSUM") as ps_q2,
                tc.tile_pool(name="bwork", bufs=4) as bwork,
                tc.tile_pool(name="bout", bufs=4) as bout,
            ):
                NDW = P + 2  # num (128) + den (2) columns per pair
                # deferred-pQ x blocks: prefetch block j at nd-block 6j (well
                # ahead of its qt-groups, emitted one per nd-block from 24)
                xq_blocks = {}

                def prefetch_xq(j):
                    sbq = QS0 + j
                    tiles = []
                    for kd in range(NKD):
                        xt_kd = xload.tile([P, SB], BF16, tag=f"xq{kd}",
                                           name=f"xq_{sbq}_{kd}", bufs=5)
                        nc.sync.dma_start(
                            out=xt_kd, in_=xt_r[:, kd, sbq * SB:(sbq + 1) * SB]
                        )
                        tiles.append(xt_kd)
                    xq_blocks[j] = tiles

                NQG = NPAIR * QSHIFT   # 28 deferred qt-groups
                QG_START = 24          # first nd-block that carries a group
                for sb2 in range(NB2):
                    if sb2 % 6 == 0 and sb2 // 6 < QSHIFT:
                        prefetch_xq(sb2 // 6)
                    g = sb2 - QG_START
                    if 0 <= g < NQG:
                        j, qt = divmod(g, NPAIR)
                        emit_pq_qt(xq_blocks[j], QS0 + j, qt, ps_q2)
                    # two psum tiles of 2 pairs each: 2*130 f32 = 1040 B/bank
                    nds = [
                        ps_nd.tile([P, 2, NDW], F32, tag=f"nd{i}",
                                   name=f"nd{i}_{sb2}")
                        for i in range(2)
                    ]
                    for pr in range(NPAIR):
                        nc.tensor.matmul(
                            nds[pr // 2][:, pr % 2, :],
                            phiq_sb[:, pr, sb2 * P:(sb2 + 1) * P],
                            rhs_all[:, pr, :],
                            start=(pr % 2 == 0), stop=(pr % 2 == 1),
                        )
                    # rec = 1/(den+eps). den >= 64*8192 (phi >= 1 everywhere),
                    # so EPS=1e-6 is ~12 orders below den and vanishes in fp32
                    # rounding — skip the eps add, reciprocal straight from PSUM.
                    rec = bwork.tile([P, 2, 2, 2], F32, tag="rec")
                    for i in range(2):
                        nc.vector.reciprocal(rec[:, i], nds[i][:, :, P:P + 2])
                    o_t = bout.tile([P, 2 * NPAIR, HD], F32, tag="o")
                    # broadcast multiply: out[s, h, v] = num * rec[s, h]
                    # one 4D-AP op per nd tile (2 pairs each)
                    for i in range(2):
                        nc.vector.tensor_tensor(
                            o_t[:, 4 * i:4 * i + 4, :].rearrange(
                                "p (q j) v -> p q j v", q=2
                            ),
                            nds[i][:, :, 0:P].rearrange(
                                "p q (j v) -> p q j v", v=HD
                            ),
                            rec[:, i, :, :, None].to_broadcast((P, 2, 2, HD)),
                            mybir.AluOpType.mult,
                        )
                    nc.sync.dma_start(
                        out=out[sb2 * P:(sb2 + 1) * P, :],
                        in_=o_t[:, :, :].rearrange("p h v -> p (h v)"),
                    )
    nc.finalize()
    return nc


def _get_nc():
    if "nc" not in _CACHE:
        _CACHE["nc"] = _build_nc()
    return _CACHE["nc"]


def _prep_inputs(x, W_qkv, b_qkv, W_p, b_p):
    """Host-side sharding + weight folding (fp64 fold, bf16 shipping).
    Biases are zero by construction in setup_inputs(); the fold keeps the
    zero bias exact."""
    x = np.asarray(x, dtype=np.float32)
    W_qkv = np.asarray(W_qkv, dtype=np.float32)
    W_p = np.asarray(W_p, dtype=np.float32)
    bf16 = ml_dtypes.bfloat16

    Wq = W_qkv[0:D]
    Wk = W_qkv[D:2 * D]
    Wv = W_qkv[2 * D:3 * D]
    Wp64 = W_p.astype(np.float64)

    xt_b = [np.ascontiguousarray(x[b].T.astype(bf16)) for b in range(B)]

    in_maps = []
    for core in range(NCORES):
        b = core % B
        g = core // B
        rows = slice(g * CV, (g + 1) * CV)
        Wq_g = Wq[rows].astype(np.float64).reshape(HG, HD, D)
        Wk_g = Wk[rows].astype(np.float64).reshape(HG, HD, D)
        # fold the shared AD-projection into the qkv projection
        wqp_g = np.einsum("ah,ghd->gad", Wp64, Wq_g).reshape(CH, D)
        wkp_g = np.einsum("ah,ghd->gad", Wp64, Wk_g).reshape(CH, D)
        wqpT = np.ascontiguousarray(wqp_g.T.astype(bf16))
        wkvT = np.ascontiguousarray(
            np.concatenate([wkp_g.T.astype(np.float32),
                            Wv[rows].T.astype(np.float32)], axis=1).astype(bf16)
        )
        in_maps.append({"xt": xt_b[b], "wqp": wqpT, "wkv": wkvT})
    return in_maps


def kernel(x, W_qkv, b_qkv, W_p, b_p):
    global LAST_RESULTS
    in_maps = _prep_inputs(x, W_qkv, b_qkv, W_p, b_p)
    res = run_bass_kernel_spmd(_get_nc(), in_maps, core_ids=list(range(NCORES)))
    LAST_RESULTS = res
    out_full = np.empty((B, S, D), np.float32)
    for core in range(NCORES):
        b = core % B
        g = core // B
        out_full[b, :, g * CV:(g + 1) * CV] = res.results[core]["out"]
    return out_full


# revision 37
# speedup vs baseline: 1.0363x; 1.0086x over previous
"""Linear-attention (sparse_attention) Trainium2 Bass kernel.

Problem: nn_Attention_Linear_25709674234652
  B=4, S=8192, D=1024, H=16 heads, HD=64, AD=64 (approx dim), EPS=1e-6

  qkv = x @ W_qkv.T (+0)          [B,S,3D]
  per head: pQ = Q @ W_p.T, pK = K @ W_p.T, phi(u) = sqrt(1+u^2)
  KTV = phi_K^T @ V  [AD,HD],  k_sum = sum_s phi_K
  out = (phi_Q @ KTV) / (phi_Q @ k_sum + eps)

Sharding: 8 cores = 4 batches x 2 head-groups (8 heads each). Each core is
fully independent (no collectives).

Host-side tricks:
  - W_p @ W_q and W_p @ W_k are folded into single projection matrices, so
    the device computes pQ / pK directly from x; Q and K never exist.
  - x is passed transposed (x^T) so the contraction dim D is already on
    partitions; no on-chip transposes anywhere.
  - inputs cast to bf16 on host (fp32 matmul on TRN2 costs ~4x bf16);
    fp32 accumulation in PSUM. Validated rel err ~3.4e-3.

Device structure:
  - pass A (per 512-col s-block): pQ^T feature-major -> phi -> bf16
    phi_Q kept RESIDENT in SBUF (8 MiB); pK|V row-major -> phi(pK), V
    -> KTV accumulated over all of S in PSUM (k_sum rides along as a
    ones-column appended to V). KTV matmuls are emitted ~2 blocks late
    so the in-order PE never waits on the ACT phi chain.
  - pass B (per 128-row s-block): one N=130 matmul per head-pair against
    block-diagonal KTV + k_sum columns (num and den in one shot),
    reciprocal + broadcast multiply on DVE, store fp32. The last 5
    s-blocks' pQ matmul groups are deferred into pass B to fill its
    otherwise-idle PE (they have no KTV dependency).

Measured on HW: ~412 us exec (core 0 NTFF), rel err 3.4e-3 vs fp32 ref.
PE-bound: bf16 projection floor is ~330 us of the ~390 us PE-busy time.
"""

import numpy as np
import ml_dtypes

import concourse.bass as bass
import concourse.tile as tile
from concourse import bacc, mybir
from concourse.bass_utils import run_bass_kernel_spmd

# ---- problem dims (hardcoded per spec) ----
B, S, D = 4, 8192, 1024
H, HD, AD = 16, 64, 64
EPS = 1e-6
NCORES = 8
HG = H // 2          # heads per core = 8
CH = HG * AD         # phi channels per core = 512
CV = HG * HD         # value channels per core = 512
P = 128
NKD = D // P         # 8 contraction tiles
SB = 512             # pass-A s-block
NSB = S // SB        # 16
NPAIR = CH // P      # 4 head-pairs per core
NB2 = S // P         # 64 pass-B s-blocks
F32 = mybir.dt.float32
BF16 = mybir.dt.bfloat16

_CACHE = {}
LAST_RESULTS = None  # BassKernelResults of most recent run (for profiling)


def _build_nc():
    nc = bacc.Bacc()
    AF = mybir.ActivationFunctionType

    xt = nc.dram_tensor("xt", [D, S], BF16, kind="ExternalInput")
    wqp = nc.dram_tensor("wqp", [D, CH], BF16, kind="ExternalInput")
    wkv = nc.dram_tensor("wkv", [D, CH + CV], BF16, kind="ExternalInput")
    out = nc.dram_tensor("out", [S, CV], F32, kind="ExternalOutput")

    xt_r = xt.rearrange("(kd p) s -> p kd s", p=P)
    wqp_r = wqp.rearrange("(kd p) c -> p kd c", p=P)
    wkv_r = wkv.rearrange("(kd p) c -> p kd c", p=P)

    with tile.TileContext(nc) as tc:
        with (
            tc.tile_pool(name="singles", bufs=1) as singles,
            tc.tile_pool(name="xload", bufs=2) as xload,
            tc.tile_pool(name="sqp", bufs=3) as sqpool,
            tc.tile_pool(name="phikp", bufs=6) as phikpool,
            tc.tile_pool(name="vp", bufs=6) as vpool,
        ):
            # startup critical path: per-kd DMAs so the first matmul (needs
            # only x[kd0] + wqp[kd0]) starts after ~0.4 MiB, not ~4 MiB
            def load_x_block(sb):
                tiles = []
                for kd in range(NKD):
                    xt_kd = xload.tile([P, SB], BF16, tag=f"x{kd}",
                                       name=f"x_{sb}_{kd}")
                    nc.sync.dma_start(
                        out=xt_kd, in_=xt_r[:, kd, sb * SB:(sb + 1) * SB]
                    )
                    tiles.append(xt_kd)
                return tiles

            # interleave x[kd] / wqp[kd] so the kd=0 matmul's deps drain first
            w_qp = singles.tile([P, NKD, CH], BF16)
            w_kv = singles.tile([P, NKD, CH + CV], BF16)
            x_first = []
            for kd in range(NKD):
                xt_kd = xload.tile([P, SB], BF16, tag=f"x{kd}", name=f"x_0_{kd}")
                nc.sync.dma_start(out=xt_kd, in_=xt_r[:, kd, 0:SB])
                x_first.append(xt_kd)
                nc.sync.dma_start(out=w_qp[:, kd], in_=wqp_r[:, kd])
            for kd in range(NKD):
                nc.sync.dma_start(out=w_kv[:, kd], in_=wkv_r[:, kd])
            # phi_Q^T resident: [128, 4 q-tiles, S] bf16 = 64 KiB/partition
            phiq_sb = singles.tile([P, NPAIR, S], BF16)

            with (
                tc.tile_pool(name="ps_q", bufs=2, space="PSUM") as ps_q,
                tc.tile_pool(name="ps_k", bufs=2, space="PSUM") as ps_k,
                tc.tile_pool(name="ps_v", bufs=2, space="PSUM") as ps_v,
                tc.tile_pool(name="ps_acc", bufs=1, space="PSUM") as ps_acc,
            ):
                # persistent accumulators, live across the whole pass.
                # col 128 of each pair block accumulates k_sum (ones column
                # appended to V), so no separate ksum matmuls are needed.
                # 2 pairs x 129 cols = 1032 B < 2 KiB, fits one bank.
                PV1 = P + 1
                ktv_ps_ab = [
                    ps_acc.tile([P, 2, PV1], F32, tag=f"ktv{i}", name=f"ktv{i}")
                    for i in range(2)
                ]

                pending = []

                def emit_ktv(phik_t, v_t, idx):
                    first = idx == 0
                    last = idx == 4 * NSB - 1
                    for pr in range(NPAIR):
                        # [128s x 128a].T @ [128s x 129(v|1)] -> a-pair x (v|ksum)
                        # off-diagonal 64x64 blocks are cross-head garbage,
                        # masked out when copying to SBUF.
                        nc.tensor.matmul(
                            ktv_ps_ab[pr // 2][:, pr % 2, :],
                            phik_t[:, pr * P:(pr + 1) * P],
                            v_t[:, pr, :],
                            start=(first and pr % 2 == 0),
                            stop=(last and pr % 2 == 1),
                        )

                def emit_pq_qt(x_t, sb, qt, pool):
                    # one pQ^T q-tile: matmul group + phi -> resident bf16
                    pq_ps = pool.tile([P, SB], F32, tag="pq",
                                      name=f"pq_{sb}_{qt}")
                    for kd in range(NKD):
                        nc.tensor.matmul(
                            pq_ps,
                            w_qp[:, kd, qt * P:(qt + 1) * P],
                            x_t[kd],
                            start=(kd == 0),
                            stop=(kd == NKD - 1),
                        )
                    sq_t = sqpool.tile([P, SB], F32, tag="sq_q")
                    nc.scalar.square(sq_t, pq_ps)
                    nc.scalar.activation(
                        phiq_sb[:, qt, sb * SB:(sb + 1) * SB],
                        sq_t, AF.Sqrt, bias=1.0,
                    )

                def emit_pq(x_t, sb, pool):
                    for qt in range(NPAIR):
                        emit_pq_qt(x_t, sb, qt, pool)

                # the last QSHIFT blocks' pQ groups are deferred into pass B
                # (no KTV dependency): spread over pass B's TAIL at qt-group
                # granularity to keep the PE dense there — pass B's bursty
                # pattern otherwise lets HAM re-throttle the PE to 1.2 GHz
                QSHIFT = 7
                QS0 = NSB - QSHIFT
                for sb in range(NSB):
                    x_t = x_first if sb == 0 else load_x_block(sb)
                    if sb < QS0:
                        emit_pq(x_t, sb, ps_q)
                    # ---- row-major pK | V + phi + KTV/ksum accumulate ----
                    for st in range(4):
                        pk_ps = ps_k.tile([P, CH], F32, tag="pk")
                        v_ps = ps_v.tile([P, CV], F32, tag="v")
                        for kd in range(NKD):
                            lhsT = x_t[kd][:, st * P:(st + 1) * P]
                            nc.tensor.matmul(
                                pk_ps, lhsT, w_kv[:, kd, :CH],
                                start=(kd == 0), stop=(kd == NKD - 1),
                            )
                            nc.tensor.matmul(
                                v_ps, lhsT, w_kv[:, kd, CH:],
                                start=(kd == 0), stop=(kd == NKD - 1),
                            )
                        sqk_t = sqpool.tile([P, CH], F32, tag="sq_k")
                        nc.scalar.square(sqk_t, pk_ps)
                        phik_t = phikpool.tile([P, CH], BF16, tag="phik")
                        nc.scalar.activation(phik_t, sqk_t, AF.Sqrt, bias=1.0)
                        # V pairs with a ones column appended (k_sum rides the
                        # KTV matmul as output column 128)
                        v_t = vpool.tile([P, NPAIR, P + 1], BF16, tag="vsb")
                        nc.vector.tensor_copy(
                            out=v_t[:, :, 0:P],
                            in_=v_ps[:, :].rearrange("p (q v) -> p q v", v=P),
                        )
                        nc.vector.memset(v_t[:, :, P:P + 1], 1.0)
                        pending.append((phik_t, v_t, sb * 4 + st))
                        # defer KTV emission ~3 blocks so PE never waits on phi
                        while len(pending) > 3:
                            emit_ktv(*pending.pop(0))
                for item in pending:
                    emit_ktv(*item)
                pending.clear()

                # ---- KTV -> block-diag SBUF (bf16), ksum in cols 128-129 ----
                # rhs_all[:, pr] = [ktv_bd (128) | ksum_h0 col | ksum_h1 col]
                # so pass B's den rides the same matmul as num (N=130).
                rhs_all = singles.tile([P, NPAIR, P + 2], BF16)
                nc.vector.memset(rhs_all, 0.0)
                HA = AD  # 64
                for pr in range(NPAIR):
                    kps = ktv_ps_ab[pr // 2][:, pr % 2, :]
                    nc.vector.tensor_copy(
                        out=rhs_all[0:HA, pr, 0:HA], in_=kps[0:HA, 0:HA]
                    )
                    nc.vector.tensor_copy(
                        out=rhs_all[HA:P, pr, HA:P], in_=kps[HA:P, HA:P]
                    )
                    nc.vector.tensor_copy(
                        out=rhs_all[0:HA, pr, P:P + 1], in_=kps[0:HA, P:P + 1]
                    )
                    nc.vector.tensor_copy(
                        out=rhs_all[HA:P, pr, P + 1:P + 2], in_=kps[HA:P, P:P + 1]
                    )

            # ---- pass B: numerator / denominator / divide / store ----
            with (
                tc.tile_pool(name="ps_nd", bufs=3, space="PSUM") as ps_nd,
                tc.tile_pool(name="ps_q2", bufs=2, space="PSUM") as ps_q2,
                tc.tile_pool(name="bwork", bufs=4) as bwork,
                tc.tile_pool(name="bout", bufs=4) as bout,
            ):
                NDW = P + 2  # num (128) + den (2) columns per pair
                # deferred-pQ x blocks: prefetch block j at nd-block 6j (well
                # ahead of its qt-groups, emitted one per nd-block from 24)
                xq_blocks = {}

                def prefetch_xq(j):
                    sbq = QS0 + j
                    tiles = []
                    for kd in range(NKD):
                        xt_kd = xload.tile([P, SB], BF16, tag=f"xq{kd}",
                                           name=f"xq_{sbq}_{kd}", bufs=5)
                        nc.sync.dma_start(
                            out=xt_kd, in_=xt_r[:, kd, sbq * SB:(sbq + 1) * SB]
                        )
                        tiles.append(xt_kd)
                    xq_blocks[j] = tiles

                NQG = NPAIR * QSHIFT   # 28 deferred qt-groups
                QG_START = 27          # deadline-packed: group g at block 27+g

                def emit_warm_mm(n, key):
                    # dummy matmuls on resident weights into a dead psum tile:
                    # keeps the PE duty cycle high enough that HAM doesn't
                    # re-throttle to 1.2 GHz during bursty stretches
                    wp = ps_q2.tile([P, SB], F32, tag="pq", name=f"warm_{key}")
                    for k in range(n):
                        nc.tensor.matmul(
                            wp, w_qp[:, k, 0:P], w_kv[:, k, 0:SB],
                            start=(k == 0), stop=(k == n - 1),
                        )

                emit_warm_mm(4, "boundary")
                for sb2 in range(NB2):
                    if sb2 % 6 == 0 and sb2 // 6 < QSHIFT:
                        prefetch_xq(sb2 // 6)
                    g = sb2 - QG_START
                    if 0 <= g < NQG:
                        j, qt = divmod(g, NPAIR)
                        emit_pq_qt(xq_blocks[j], QS0 + j, qt, ps_q2)
                    elif g >= NQG:
                        emit_warm_mm(2, f"tail_{sb2}")
                    # two psum tiles of 2 pairs each: 2*130 f32 = 1040 B/bank
                    nds = [
                        ps_nd.tile([P, 2, NDW], F32, tag=f"nd{i}",
                                   name=f"nd{i}_{sb2}")
                        for i in range(2)
                    ]
                    for pr in range(NPAIR):
                        nc.tensor.matmul(
                            nds[pr // 2][:, pr % 2, :],
                            phiq_sb[:, pr, sb2 * P:(sb2 + 1) * P],
                            rhs_all[:, pr, :],
                            start=(pr % 2 == 0), stop=(pr % 2 == 1),
                        )
                    # rec = 1/(den+eps). den >= 64*8192 (phi >= 1 everywhere),
                    # so EPS=1e-6 is ~12 orders below den and vanishes in fp32
                    # rounding — skip the eps add, reciprocal straight from PSUM.
                    rec = bwork.tile([P, 2, 2, 2], F32, tag="rec")
                    for i in range(2):
                        nc.vector.reciprocal(rec[:, i], nds[i][:, :, P:P + 2])
                    o_t = bout.tile([P, 2 * NPAIR, HD], F32, tag="o")
                    # broadcast multiply: out[s, h, v] = num * rec[s, h]
                    # one 4D-AP op per nd tile (2 pairs each)
                    for i in range(2):
                        nc.vector.tensor_tensor(
                            o_t[:, 4 * i:4 * i + 4, :].rearrange(
                                "p (q j) v -> p q j v", q=2
                            ),
                            nds[i][:, :, 0:P].rearrange(
                                "p q (j v) -> p q j v", v=HD
                            ),
                            rec[:, i, :, :, None].to_broadcast((P, 2, 2, HD)),
                            mybir.AluOpType.mult,
                        )
                    nc.sync.dma_start(
                        out=out[sb2 * P:(sb2 + 1) * P, :],
                        in_=o_t[:, :, :].rearrange("p h v -> p (h v)"),
                    )
    nc.finalize()
    return nc


def _get_nc():
    if "nc" not in _CACHE:
        _CACHE["nc"] = _build_nc()
    return _CACHE["nc"]


def _prep_inputs(x, W_qkv, b_qkv, W_p, b_p):
    """Host-side sharding + weight folding (fp64 fold, bf16 shipping).
    Biases are zero by construction in setup_inputs(); the fold keeps the
    zero bias exact."""
    x = np.asarray(x, dtype=np.float32)
    W_qkv = np.asarray(W_qkv, dtype=np.float32)
    W_p = np.asarray(W_p, dtype=np.float32)
    bf16 = ml_dtypes.bfloat16

    Wq = W_qkv[0:D]
    Wk = W_qkv[D:2 * D]
    Wv = W_qkv[2 * D:3 * D]
    Wp64 = W_p.astype(np.float64)

    xt_b = [np.ascontiguousarray(x[b].T.astype(bf16)) for b in range(B)]

    in_maps = []
    for core in range(NCORES):
        b = core % B
        g = core // B
        rows = slice(g * CV, (g + 1) * CV)
        Wq_g = Wq[rows].astype(np.float64).reshape(HG, HD, D)
        Wk_g = Wk[rows].astype(np.float64).reshape(HG, HD, D)
        # fold the shared AD-projection into the qkv projection
        wqp_g = np.einsum("ah,ghd->gad", Wp64, Wq_g).reshape(CH, D)
        wkp_g = np.einsum("ah,ghd->gad", Wp64, Wk_g).reshape(CH, D)
        wqpT = np.ascontiguousarray(wqp_g.T.astype(bf16))
        wkvT = np.ascontiguousarray(
            np.concatenate([wkp_g.T.astype(np.float32),
                            Wv[rows].T.astype(np.float32)], axis=1).astype(bf16)
        )
        in_maps.append({"xt": xt_b[b], "wqp": wqpT, "wkv": wkvT})
    return in_maps


def kernel(x, W_qkv, b_qkv, W_p, b_p):
    global LAST_RESULTS
    in_maps = _prep_inputs(x, W_qkv, b_qkv, W_p, b_p)
    res = run_bass_kernel_spmd(_get_nc(), in_maps, core_ids=list(range(NCORES)))
    LAST_RESULTS = res
    out_full = np.empty((B, S, D), np.float32)
    for core in range(NCORES):
        b = core % B
        g = core // B
        out_full[b, :, g * CV:(g + 1) * CV] = res.results[core]["out"]
    return out_full
